# revision 13
# baseline (speedup 1.0000x reference)
"""NVFP4-style activation quantizer v3.2 on 8 TRN2 NeuronCores.

Self-contained: hardcodes shapes/sharding for x of shape (2, 2048, 4096) f32.
Data-parallel: flat tensor split into 8 contiguous shards [128 x 16384].

v3.2 replaces v2's magic/r1/predicate/select rounding (2 ACT passes + 3 DVE
TS + 1x-rate copy_predicated) with a select-free "variable magic":

  fp4 rounding of fp16 f (|f| <= 6.4) == RNE(f + T) - T where T is the
  per-element power-of-2 magic 768 * 2^max(0, e_f - 14):
    ulp(768)  = 0.5  -> the 0.5-step grid {0,.5,..,2}   for |f| < 2
    ulp(1536) = 1.0  -> the 1-step grid   {2,3,4}       for [2,4)
    ulp(3072) = 2.0  -> the 2-step grid   {4,6}         for [4,6.4]
  T via two dual-op int16 TS (class-consistent ops, verifier-checked):
    u = (bits(f) & 0x7C00) | 0x200        (bitwise, bitwise)
    T = max(u + 0x2400, 0x6200)           (arith,   arith)
  then qv = fp16(f + T); q = qv - T (both fp16 TT at 2x rate).
  Tie-away-from-zero comes from folding s0 = 1+2^-11 into the reciprocal:
  r6h = fp16(6*s0/scale), nudging |f| up so RNE ties round away.

Numerics validated in numpy vs the jax reference: L2 = 1.02e-2 (gate 2e-2).
f32-exact group amax; e4m3(amax) IS a float8e4 cast on ACT.

Engine split per tile (DVE is the bottleneck; ACT absorbs all casts/scales;
gpsimd does ONLY the SWDGE widening out-DMA - its elementwise ops are
compiler-rejected and SWDGE input DMAs were measured to inflate concurrent
DVE ops ~20% via the shared POOL/DVE SBUF port):
  sync  : DMA-in xt f32 (HWDGE)
  DVE   : am = group abs-max (tensor_reduce f32, exact)
  ACT   : xh = fp16(xt); scd = e4m3(am) pairs; r6h = fp16(6*s0/scale);
          o16 = fp16(scale/6); sx = expand pairs -> [P,2,GT,16]
  DVE   : f = fp16(xh * r6x); u/T dual TS; qv = fp16(f+T); q = qv-T;
          y = fp16(q * o16x)
  gpsimd: DMA-out y fp16 -> out f32 HBM (SWDGE widening cast, exact)
"""
import sys

sys.path.insert(0, "/opt/trn_rl_repo")

import numpy as np

import concourse.bass as bass
import concourse.bacc as bacc
import concourse.mybir as mybir
from concourse import tile
from concourse.bass_utils import run_bass_kernel_spmd

AF = mybir.ActivationFunctionType
ALU = mybir.AluOpType

N_CORES = 8
FULL_SHAPE = (2, 2048, 4096)
TOTAL = 2 * 2048 * 4096            # 16,777,216
PER_CORE = TOTAL // N_CORES        # 2,097,152
P = 128
FD = PER_CORE // P                 # 16384 free elems per partition
TILE_SIZES = [256, 512, 1024, 2048, 2560, 2560, 2560, 2560, 1792, 512]
assert sum(TILE_SIZES) == FD

S0 = float(np.float32(1.0) + np.float32(2.0 ** -11))

_cached_nc = None


def _act_recip(nc, out_ap, in_ap, scale):
    """out = fp16(1 / (in * scale)) on ACT. Bass blocks AF.Reciprocal for
    accuracy; our input has 4 significant bits so the table is exact enough
    (validated: bit-exact vs numpy for e4m3 inputs)."""
    eng = nc.scalar
    ins = [eng.lower_ap(in_ap),
           mybir.ImmediateValue(dtype=mybir.dt.float32, value=0.0),
           mybir.ImmediateValue(dtype=mybir.dt.float32, value=float(scale)),
           mybir.ImmediateValue(dtype=mybir.dt.float32, value=0.0)]
    return eng.add_instruction(
        mybir.InstActivation(
            name=nc.get_next_instruction_name(),
            func=mybir.ActivationFunctionType.Reciprocal,
            ins=ins,
            outs=[eng.lower_ap(out_ap)],
        ))


def build_nc() -> bass.Bass:
    nc = bacc.Bacc("TRN2", target_bir_lowering=False, debug=False)
    x = nc.dram_tensor("x", [P, FD], mybir.dt.float32, kind="ExternalInput")
    # output y is fp16-valued by construction (q and o16 are fp16 and the
    # final product is written as fp16); emit fp16 over HWDGE and widen
    # exactly on the host — halves HBM write traffic and drops the SWDGE
    # out-path (gpsimd rings + teardown drains) entirely.
    out = nc.dram_tensor("out", [P, FD], mybir.dt.float16, kind="ExternalOutput")

    i16 = mybir.dt.int16
    f16 = mybir.dt.float16
    f32 = mybir.dt.float32
    f8 = mybir.dt.float8e4

    with tile.TileContext(nc) as tc:
        with tc.tile_pool(name="xt", bufs=3) as xt_pool, \
             tc.tile_pool(name="xh", bufs=4) as xh_pool, \
             tc.tile_pool(name="sx", bufs=4) as sx_pool, \
             tc.tile_pool(name="f", bufs=2) as f_pool, \
             tc.tile_pool(name="tf", bufs=2) as tf_pool, \
             tc.tile_pool(name="qv", bufs=2) as qv_pool, \
             tc.tile_pool(name="q", bufs=2) as q_pool, \
             tc.tile_pool(name="y", bufs=2) as y_pool, \
             tc.tile_pool(name="small", bufs=2) as small:
            T = len(TILE_SIZES)
            offs = [sum(TILE_SIZES[:i]) for i in range(T)]
            st = {}

            def stage_in(t):
                FT = TILE_SIZES[t]
                GT = FT // 16
                sl = slice(offs[t], offs[t] + FT)
                xt = xt_pool.tile([P, FT], f32, tag="xt", name="xt")
                nc.sync.dma_start(out=xt[:], in_=x[:, sl])
                # group abs-max (f32-exact) on DVE (queue tail: waits DMA)
                am = small.tile([P, GT], f32, tag="am", name="am")
                nc.vector.tensor_reduce(
                    am[:], xt[:].rearrange("p (g s) -> p g s", s=16),
                    axis=mybir.AxisListType.X, op=ALU.max,
                    apply_absolute_value=True)
                # ACT: fp16 cast, e4m3 scale (pair-duplicated), derived scales
                xh = xh_pool.tile([P, FT], f16, tag="xh", name="xh")
                nc.scalar.activation(xh[:], xt[:], AF.Copy)
                scd = small.tile([P, GT, 2], f8, tag="scd", name="scd")
                nc.scalar.activation(
                    scd[:], am[:].unsqueeze(2).broadcast_to((P, GT, 2)),
                    AF.Copy)
                s2 = small.tile([P, 2, GT, 2], f16, tag="s2", name="s2")
                sx = sx_pool.tile([P, 2, GT, 16], f16, tag="sx", name="sx")
                # r6 half first: it gates the f-multiply of this tile, so
                # expand it before deriving/expanding the o16 half
                _act_recip(nc, s2[:, 0], scd[:], 1.0 / (6.0 * S0))
                nc.scalar.activation(
                    sx[:, 0:1].bitcast(f32),
                    s2[:, 0:1].bitcast(f32).broadcast_to((P, 1, GT, 8)),
                    AF.Copy)
                nc.scalar.activation(s2[:, 1], scd[:], AF.Copy,
                                     scale=1.0 / 6.0)
                nc.scalar.activation(
                    sx[:, 1:2].bitcast(f32),
                    s2[:, 1:2].bitcast(f32).broadcast_to((P, 1, GT, 8)),
                    AF.Copy)
                st[t] = {"xh": xh, "sx": sx}

            def stage_rest(t):
                FT = TILE_SIZES[t]
                sl = slice(offs[t], offs[t] + FT)
                d = st[t]
                # f = fp16(xh * r6x)   (fp16 TT, 2x mode)
                f = f_pool.tile([P, FT], f16, tag="f", name="f")
                nc.vector.tensor_tensor(
                    f[:].rearrange("p (g s) -> p g s", s=16),
                    d["xh"][:].rearrange("p (g s) -> p g s", s=16),
                    d["sx"][:, 0], ALU.mult)
                # per-element magic T = 768 * 2^max(0, e_f - 14):
                #   u = (bits & 0x7C00) | 0x200 ; T = max(u + 0x2400, 0x6200)
                tf = tf_pool.tile([P, FT], f16, tag="tf", name="tf")
                nc.vector.tensor_scalar(
                    tf[:].bitcast(i16), f[:].bitcast(i16), 0x7C00, 0x200,
                    ALU.bitwise_and, ALU.bitwise_or)
                nc.vector.tensor_scalar(
                    tf[:].bitcast(i16), tf[:].bitcast(i16), 0x2400, 0x6200,
                    ALU.add, ALU.max)
                # qv = fp16(f + T)  (rounds to the per-element grid), q = qv-T
                qv = qv_pool.tile([P, FT], f16, tag="qv", name="qv")
                nc.vector.tensor_tensor(qv[:], f[:], tf[:], ALU.add)
                q = q_pool.tile([P, FT], f16, tag="q", name="q")
                nc.vector.tensor_tensor(q[:], qv[:], tf[:], ALU.subtract)
                # y = fp16(q * o16x)
                y = y_pool.tile([P, FT], f16, tag="y", name="y")
                nc.vector.tensor_tensor(
                    y[:].rearrange("p (g s) -> p g s", s=16),
                    q[:].rearrange("p (g s) -> p g s", s=16),
                    d["sx"][:, 1], ALU.mult)
                # fp16 out over HWDGE on the ACT queue (no cast; host widens
                # exactly). Keeping outputs OFF the sync queue stops out-DMA
                # y-waits from head-blocking later input DMA issues, so
                # inputs always run the full pipeline depth ahead; ACT's
                # scale chain has 2 tiles of slack to absorb the wait.
                nc.scalar.dma_start(out=out[:, sl], in_=y[:])
                del st[t]

            # 3-deep software pipeline: in(i) runs two tiles ahead of
            # rest(i-2), giving the ACT scale chain (reduce -> e4m3 ->
            # recip -> expand) two tile-times of slack before the
            # f-multiply consumes it; the DVE reduce sits at the queue
            # tail so fresh-DMA waits never block older tiles' math.
            for i in range(T + 2):
                if 0 <= i - 2 < T:
                    stage_rest(i - 2)
                if i < T:
                    stage_in(i)
    nc.compile()
    return nc


def _get_nc() -> bass.Bass:
    global _cached_nc
    if _cached_nc is None:
        _cached_nc = build_nc()
    return _cached_nc


def run(x: np.ndarray, trace: bool = False, **kw):
    """Shard, run SPMD on 8 cores, gather. Returns (out_full, BassKernelResults)."""
    x_flat = np.ascontiguousarray(np.asarray(x, dtype=np.float32)).reshape(-1)
    in_maps = [
        {"x": x_flat[i * PER_CORE:(i + 1) * PER_CORE].reshape(P, FD)}
        for i in range(N_CORES)
    ]
    nc = _get_nc()
    res = run_bass_kernel_spmd(nc, in_maps, core_ids=list(range(N_CORES)),
                               trace=trace, **kw)
    out = np.empty(TOTAL, dtype=np.float32)
    for i in range(N_CORES):
        # exact widening: y values are fp16 by construction
        out[i * PER_CORE:(i + 1) * PER_CORE] = (
            res.results[i]["out"].astype(np.float32).reshape(-1))
    return out.reshape(FULL_SHAPE), res


def kernel(x: np.ndarray) -> np.ndarray:
    out, _ = run(x, trace=False)
    return out


# revision 14
# speedup vs baseline: 1.1827x; 1.1827x over previous
"""NVFP4-style activation quantizer v3.2 on 8 TRN2 NeuronCores.

Self-contained: hardcodes shapes/sharding for x of shape (2, 2048, 4096) f32.
Data-parallel: flat tensor split into 8 contiguous shards [128 x 16384].

v3.2 replaces v2's magic/r1/predicate/select rounding (2 ACT passes + 3 DVE
TS + 1x-rate copy_predicated) with a select-free "variable magic":

  fp4 rounding of fp16 f (|f| <= 6.4) == RNE(f + T) - T where T is the
  per-element power-of-2 magic 768 * 2^max(0, e_f - 14):
    ulp(768)  = 0.5  -> the 0.5-step grid {0,.5,..,2}   for |f| < 2
    ulp(1536) = 1.0  -> the 1-step grid   {2,3,4}       for [2,4)
    ulp(3072) = 2.0  -> the 2-step grid   {4,6}         for [4,6.4]
  T via two dual-op int16 TS (class-consistent ops, verifier-checked):
    u = (bits(f) & 0x7C00) | 0x200        (bitwise, bitwise)
    T = max(u + 0x2400, 0x6200)           (arith,   arith)
  then qv = fp16(f + T); q = qv - T (both fp16 TT at 2x rate).
  Tie-away-from-zero comes from folding s0 = 1+2^-11 into the reciprocal:
  r6h = fp16(6*s0/scale), nudging |f| up so RNE ties round away.

Numerics validated in numpy vs the jax reference: L2 = 1.02e-2 (gate 2e-2).
f32-exact group amax; e4m3(amax) IS a float8e4 cast on ACT.

Engine split per tile (DVE is the bottleneck; ACT absorbs all casts/scales;
gpsimd does ONLY the SWDGE widening out-DMA - its elementwise ops are
compiler-rejected and SWDGE input DMAs were measured to inflate concurrent
DVE ops ~20% via the shared POOL/DVE SBUF port):
  sync  : DMA-in xt f32 (HWDGE)
  DVE   : am = group abs-max (tensor_reduce f32, exact)
  ACT   : xh = fp16(xt); scd = e4m3(am) pairs; r6h = fp16(6*s0/scale);
          o16 = fp16(scale/6); sx = expand pairs -> [P,2,GT,16]
  DVE   : f = fp16(xh * r6x); u/T dual TS; qv = fp16(f+T); q = qv-T;
          y = fp16(q * o16x)
  gpsimd: DMA-out y fp16 -> out f32 HBM (SWDGE widening cast, exact)
"""
import sys

sys.path.insert(0, "/opt/trn_rl_repo")

import numpy as np

import concourse.bass as bass
import concourse.bacc as bacc
import concourse.mybir as mybir
from concourse import tile
from concourse.bass_utils import run_bass_kernel_spmd

AF = mybir.ActivationFunctionType
ALU = mybir.AluOpType

N_CORES = 8
FULL_SHAPE = (2, 2048, 4096)
TOTAL = 2 * 2048 * 4096            # 16,777,216
PER_CORE = TOTAL // N_CORES        # 2,097,152
P = 128
FD = PER_CORE // P                 # 16384 free elems per partition
TILE_SIZES = [256, 512, 1024, 2048, 2560, 2560, 2560, 2560, 1792, 512]
assert sum(TILE_SIZES) == FD

S0 = float(np.float32(1.0) + np.float32(2.0 ** -11))

_cached_nc = None


def _act_recip(nc, out_ap, in_ap, scale):
    """out = fp16(1 / (in * scale)) on ACT. Bass blocks AF.Reciprocal for
    accuracy; our input has 4 significant bits so the table is exact enough
    (validated: bit-exact vs numpy for e4m3 inputs)."""
    eng = nc.scalar
    ins = [eng.lower_ap(in_ap),
           mybir.ImmediateValue(dtype=mybir.dt.float32, value=0.0),
           mybir.ImmediateValue(dtype=mybir.dt.float32, value=float(scale)),
           mybir.ImmediateValue(dtype=mybir.dt.float32, value=0.0)]
    return eng.add_instruction(
        mybir.InstActivation(
            name=nc.get_next_instruction_name(),
            func=mybir.ActivationFunctionType.Reciprocal,
            ins=ins,
            outs=[eng.lower_ap(out_ap)],
        ))


def build_nc() -> bass.Bass:
    nc = bacc.Bacc("TRN2", target_bir_lowering=False, debug=False)
    x = nc.dram_tensor("x", [P, FD], mybir.dt.float32, kind="ExternalInput")
    # output y is fp16-valued by construction (q and o16 are fp16 and the
    # final product is written as fp16); emit fp16 over HWDGE and widen
    # exactly on the host — halves HBM write traffic and drops the SWDGE
    # out-path (gpsimd rings + teardown drains) entirely.
    out = nc.dram_tensor("out", [P, FD], mybir.dt.float16, kind="ExternalOutput")

    i16 = mybir.dt.int16
    f16 = mybir.dt.float16
    f32 = mybir.dt.float32
    f8 = mybir.dt.float8e4

    with tile.TileContext(nc) as tc:
        with tc.tile_pool(name="xt", bufs=3) as xt_pool, \
             tc.tile_pool(name="xh", bufs=4) as xh_pool, \
             tc.tile_pool(name="sx", bufs=4) as sx_pool, \
             tc.tile_pool(name="f", bufs=2) as f_pool, \
             tc.tile_pool(name="tf", bufs=2) as tf_pool, \
             tc.tile_pool(name="qv", bufs=2) as qv_pool, \
             tc.tile_pool(name="q", bufs=2) as q_pool, \
             tc.tile_pool(name="y", bufs=2) as y_pool, \
             tc.tile_pool(name="small", bufs=2) as small:
            T = len(TILE_SIZES)
            offs = [sum(TILE_SIZES[:i]) for i in range(T)]
            st = {}

            def stage_in(t):
                FT = TILE_SIZES[t]
                GT = FT // 16
                sl = slice(offs[t], offs[t] + FT)
                xt = xt_pool.tile([P, FT], f32, tag="xt", name="xt")
                nc.sync.dma_start(out=xt[:], in_=x[:, sl])
                # group abs-max (f32-exact) on DVE (queue tail: waits DMA)
                am = small.tile([P, GT], f32, tag="am", name="am")
                nc.vector.tensor_reduce(
                    am[:], xt[:].rearrange("p (g s) -> p g s", s=16),
                    axis=mybir.AxisListType.X, op=ALU.max,
                    apply_absolute_value=True)
                # ACT: fp16 cast, e4m3 scale (pair-duplicated), derived scales
                xh = xh_pool.tile([P, FT], f16, tag="xh", name="xh")
                nc.scalar.activation(xh[:], xt[:], AF.Copy)
                scd = small.tile([P, GT, 2], f8, tag="scd", name="scd")
                nc.scalar.activation(
                    scd[:], am[:].unsqueeze(2).broadcast_to((P, GT, 2)),
                    AF.Copy)
                s2 = small.tile([P, 2, GT, 2], f16, tag="s2", name="s2")
                sx = sx_pool.tile([P, 2, GT, 16], f16, tag="sx", name="sx")
                # r6 half first: it gates the f-multiply of this tile, so
                # expand it before deriving/expanding the o16 half
                _act_recip(nc, s2[:, 0], scd[:], 1.0 / (6.0 * S0))
                nc.scalar.activation(
                    sx[:, 0:1].bitcast(f32),
                    s2[:, 0:1].bitcast(f32).broadcast_to((P, 1, GT, 8)),
                    AF.Copy)
                nc.scalar.activation(s2[:, 1], scd[:], AF.Copy,
                                     scale=1.0 / 6.0)
                nc.scalar.activation(
                    sx[:, 1:2].bitcast(f32),
                    s2[:, 1:2].bitcast(f32).broadcast_to((P, 1, GT, 8)),
                    AF.Copy)
                st[t] = {"xh": xh, "sx": sx}

            def stage_rest(t):
                FT = TILE_SIZES[t]
                sl = slice(offs[t], offs[t] + FT)
                d = st[t]
                # f = fp16(xh * r6x)   (fp16 TT, 2x mode)
                f = f_pool.tile([P, FT], f16, tag="f", name="f")
                nc.vector.tensor_tensor(
                    f[:].rearrange("p (g s) -> p g s", s=16),
                    d["xh"][:].rearrange("p (g s) -> p g s", s=16),
                    d["sx"][:, 0], ALU.mult)
                # per-element magic T = 768 * 2^max(0, e_f - 14):
                #   u = (bits & 0x7C00) | 0x200 ; T = max(u + 0x2400, 0x6200)
                tf = tf_pool.tile([P, FT], f16, tag="tf", name="tf")
                nc.vector.tensor_scalar(
                    tf[:].bitcast(i16), f[:].bitcast(i16), 0x7C00, 0x200,
                    ALU.bitwise_and, ALU.bitwise_or)
                nc.vector.tensor_scalar(
                    tf[:].bitcast(i16), tf[:].bitcast(i16), 0x2400, 0x6200,
                    ALU.add, ALU.max)
                # qv = fp16(f + T)  (rounds to the per-element grid), q = qv-T
                qv = qv_pool.tile([P, FT], f16, tag="qv", name="qv")
                nc.vector.tensor_tensor(qv[:], f[:], tf[:], ALU.add)
                q = q_pool.tile([P, FT], f16, tag="q", name="q")
                nc.vector.tensor_tensor(q[:], qv[:], tf[:], ALU.subtract)
                # y = fp16(q * o16x)
                y = y_pool.tile([P, FT], f16, tag="y", name="y")
                nc.vector.tensor_tensor(
                    y[:].rearrange("p (g s) -> p g s", s=16),
                    q[:].rearrange("p (g s) -> p g s", s=16),
                    d["sx"][:, 1], ALU.mult)
                # fp16 out over HWDGE on sync (no cast; host widens exactly).
                # NOTE: tried the ACT queue instead (to keep out-DMA y-waits
                # from delaying input DMA issues) — regressed 83.2 -> 98.8us:
                # the y-wait head-blocks ACT's scale chain far worse.
                nc.sync.dma_start(out=out[:, sl], in_=y[:])
                del st[t]

            # 3-deep software pipeline: in(i) runs two tiles ahead of
            # rest(i-2), giving the ACT scale chain (reduce -> e4m3 ->
            # recip -> expand) two tile-times of slack before the
            # f-multiply consumes it; the DVE reduce sits at the queue
            # tail so fresh-DMA waits never block older tiles' math.
            for i in range(T + 2):
                if 0 <= i - 2 < T:
                    stage_rest(i - 2)
                if i < T:
                    stage_in(i)
    nc.compile()
    return nc


def _get_nc() -> bass.Bass:
    global _cached_nc
    if _cached_nc is None:
        _cached_nc = build_nc()
    return _cached_nc


def run(x: np.ndarray, trace: bool = False, **kw):
    """Shard, run SPMD on 8 cores, gather. Returns (out_full, BassKernelResults)."""
    x_flat = np.ascontiguousarray(np.asarray(x, dtype=np.float32)).reshape(-1)
    in_maps = [
        {"x": x_flat[i * PER_CORE:(i + 1) * PER_CORE].reshape(P, FD)}
        for i in range(N_CORES)
    ]
    nc = _get_nc()
    res = run_bass_kernel_spmd(nc, in_maps, core_ids=list(range(N_CORES)),
                               trace=trace, **kw)
    out = np.empty(TOTAL, dtype=np.float32)
    for i in range(N_CORES):
        # exact widening: y values are fp16 by construction
        out[i * PER_CORE:(i + 1) * PER_CORE] = (
            res.results[i]["out"].astype(np.float32).reshape(-1))
    return out.reshape(FULL_SHAPE), res


def kernel(x: np.ndarray) -> np.ndarray:
    out, _ = run(x, trace=False)
    return out
